# revision 10
# baseline (speedup 1.0000x reference)
# Trainium2 Bass kernel for nn_Attention_63900523430102.
#
# Reference computes, for q,k,v of shape (S=2048, B=4, D=1024):
#   xq = q @ wq.T, xk = k @ wk.T, xv = v @ wv.T  (per-head split, hd=64, H=16)
#   xq, xk = rope(xq), rope(xk)
#   scores = softmax(xq . xk / sqrt(hd)) ; out = (scores @ xv) @ wo.T
#
# Sharding: 8 cores = 4 batches x 2 head-groups (8 heads each).  Each core
# gets host-pretransposed operands so that on-device:
#   xqT/xkT [hd, S] come straight out of the projection matmuls,
#   scoresT [l, s] needs no transposes, softmax denominator comes from a
#   ones-column appended to xv (M=65 PV matmul), and the wo partial products
#   are summed pairwise on the host.
import sys
import os

sys.path.insert(0, "/opt/trn_rl_repo")

import numpy as np
import ml_dtypes

import concourse.bass as bass
import concourse.bacc as bacc
import concourse.mybir as mybir
import concourse.tile as tile
from concourse.bass_utils import run_bass_kernel_spmd


def _shim_axon_hooks():
    """Provide antenv.axon_hooks (NTFF profile hook) if the image lacks it.

    run_bass_kernel_spmd(trace=True) under axon imports
    antenv.axon_hooks.get_axon_ntff_profile_hook; this container's antenv
    package doesn't ship it.  Recreate the ctypes-based hook against the
    axon PJRT .so (same contract as trn_boot._ntff_profile_via_ctypes).
    """
    try:
        from antenv.axon_hooks import get_axon_ntff_profile_hook  # noqa: F401
        return
    except ImportError:
        pass
    import types
    import ctypes
    import contextlib

    so_path = "/opt/axon/libaxon_pjrt.so"
    hook = None
    if os.path.exists(so_path):
        lib = ctypes.CDLL(so_path)
        if hasattr(lib, "axon_start_nrt_profile"):
            lib.axon_start_nrt_profile.argtypes = [
                ctypes.POINTER(ctypes.c_int64), ctypes.c_size_t]
            lib.axon_start_nrt_profile.restype = ctypes.c_int64
            lib.axon_stop_nrt_profile.argtypes = [ctypes.c_char_p]
            lib.axon_stop_nrt_profile.restype = ctypes.c_int64

            @contextlib.contextmanager
            def hook(output_dir, device_ids):
                import jax
                jax.devices()
                if device_ids:
                    ids = (ctypes.c_int64 * len(device_ids))(*device_ids)
                    rc = lib.axon_start_nrt_profile(ids, len(device_ids))
                else:
                    rc = lib.axon_start_nrt_profile(None, 0)
                if rc != 0:
                    raise RuntimeError(f"axon_start_nrt_profile rc={rc}")
                try:
                    yield
                finally:
                    n = lib.axon_stop_nrt_profile(str(output_dir).encode())
                    print(f"ntff profile: {n} file(s) -> {output_dir}",
                          file=sys.stderr)

    mod = types.ModuleType("antenv.axon_hooks")
    mod.get_axon_ntff_profile_hook = lambda: hook
    mod.set_axon_ntff_profile_hook = lambda h: None
    sys.modules["antenv.axon_hooks"] = mod


_shim_axon_hooks()

S = 2048          # seq len (both query s and key l)
D = 1024          # d_model
B = 4             # batch
HLOC = 8          # heads per core
HD = 64           # head dim
E = HLOC * HD     # 512, local e-width per core
NCORES = 8
NPAIR = 4         # head pairs per core (2 heads stacked -> 128 partitions)
NSB = 4           # s blocks of 512
NLT = 16          # l tiles of 128
ND = 8            # d_model tiles of 128
NST = 16          # s tiles of 128 (wo phase)

BF16 = mybir.dt.bfloat16
F32 = mybir.dt.float32
NPBF16 = ml_dtypes.bfloat16

# exp groups: l-tiles chunked so each ACT call covers several PSUM banks
GROUPS = [(0, 3), (3, 3), (6, 3), (9, 3), (12, 3), (15, 1)]

_PROG = None
LAST_RESULT = None


def _emit(nc, tc, aps):
    qT, kT, vT, wqT, wkT, wvT, woT, ctab, stab, out = aps
    Exp = mybir.ActivationFunctionType.Exp
    swap_mask = [i ^ 1 for i in range(32)]

    from contextlib import ExitStack

    with ExitStack() as stk:
        consts = stk.enter_context(tc.tile_pool(name="consts", bufs=1))
        persist = stk.enter_context(tc.tile_pool(name="persist", bufs=1))

        # ---- persistent constants ----
        # DMA queue split: q-side + ct + out on sync (HWDGE), everything the
        # v/k-side needs on gpsimd (SWDGE), ordered by first use.
        wq_t, wk_t, wv_t = [], [], []
        for d in range(ND):
            t = consts.tile([128, E], BF16, tag=f"wq{d}", name=f"wq{d}")
            nc.sync.dma_start(out=t, in_=wqT[d * 128:(d + 1) * 128, :])
            wq_t.append(t)
        for d in range(ND):
            t = consts.tile([128, E], BF16, tag=f"wv{d}", name=f"wv{d}")
            nc.gpsimd.dma_start(out=t, in_=wvT[d * 128:(d + 1) * 128, :])
            wv_t.append(t)
        for d in range(ND):
            t = consts.tile([128, E], BF16, tag=f"wk{d}", name=f"wk{d}")
            nc.gpsimd.dma_start(out=t, in_=wkT[d * 128:(d + 1) * 128, :])
            wk_t.append(t)
        ct_t = consts.tile([128, S], F32, tag="ct", name="ct")
        nc.sync.dma_start(out=ct_t, in_=ctab[:, :])
        st_t = consts.tile([128, S], F32, tag="st", name="st")
        nc.gpsimd.dma_start(out=st_t, in_=stab[:, :])
        wo_t = []
        for e in range(NPAIR):
            t = consts.tile([128, D], BF16, tag=f"wo{e}", name=f"wo{e}")
            nc.sync.dma_start(out=t, in_=woT[e * 128:(e + 1) * 128, :])
            wo_t.append(t)

        # ---- persistent activations ----
        xq_sb = [persist.tile([128, S], BF16, tag=f"xq{p}", name=f"xq{p}")
                 for p in range(NPAIR)]
        xk_sb = [persist.tile([128, S], BF16, tag=f"xk{p}", name=f"xk{p}")
                 for p in range(NPAIR)]
        # xv with a ones column per head: head h occupies cols 65h..65h+64
        xv_sb = [persist.tile([128, HLOC * (HD + 1)], BF16,
                              tag=f"xv{lt}", name=f"xv{lt}")
                 for lt in range(NLT)]
        attT = [persist.tile([128, S], BF16, tag=f"att{p}", name=f"att{p}")
                for p in range(NPAIR)]

        # =============== Phase A: projections + rope ===============
        with ExitStack() as phase_a:
            qk_pool = phase_a.enter_context(tc.tile_pool(name="qk", bufs=20))
            rope_pool = phase_a.enter_context(tc.tile_pool(name="rope", bufs=3))
            vt_pool = phase_a.enter_context(tc.tile_pool(name="vt", bufs=8))
            psA = phase_a.enter_context(
                tc.tile_pool(name="psA", bufs=2, space="PSUM"))
            psV = phase_a.enter_context(
                tc.tile_pool(name="psV", bufs=2, space="PSUM"))

            def rope(ps, dst, sb):
                cols = slice(sb * 512, (sb + 1) * 512)
                xr = rope_pool.tile([128, 512], F32, tag="xr", name="xr")
                nc.vector.tensor_copy(xr, ps)
                t1 = rope_pool.tile([128, 512], F32, tag="t1", name="t1")
                nc.vector.tensor_mul(t1, xr, ct_t[:, cols])
                sw = rope_pool.tile([128, 512], F32, tag="sw", name="sw")
                nc.vector.stream_shuffle(sw, xr, swap_mask)
                t2 = rope_pool.tile([128, 512], F32, tag="t2", name="t2")
                nc.vector.tensor_mul(t2, sw, st_t[:, cols])
                nc.vector.tensor_add(dst[:, cols], t1, t2)

            for sb in range(NSB):
                scol = slice(sb * 512, (sb + 1) * 512)
                qts, kts = [], []
                for d in range(ND):
                    qt = qk_pool.tile([128, 512], BF16, tag="qk", name="qt")
                    nc.sync.dma_start(out=qt, in_=qT[d * 128:(d + 1) * 128, scol])
                    qts.append(qt)
                    kt = qk_pool.tile([128, 512], BF16, tag="qk", name="kt")
                    nc.gpsimd.dma_start(out=kt, in_=kT[d * 128:(d + 1) * 128, scol])
                    kts.append(kt)
                for p in range(NPAIR):
                    pcol = slice(p * 128, (p + 1) * 128)
                    xq_ps = psA.tile([128, 512], F32, tag="xqps", name="xqps")
                    for d in range(ND):
                        nc.tensor.matmul(xq_ps, lhsT=wq_t[d][:, pcol],
                                         rhs=qts[d],
                                         start=(d == 0), stop=(d == ND - 1))
                    rope(xq_ps, xq_sb[p], sb)
                    xk_ps = psA.tile([128, 512], F32, tag="xkps", name="xkps")
                    for d in range(ND):
                        nc.tensor.matmul(xk_ps, lhsT=wk_t[d][:, pcol],
                                         rhs=kts[d],
                                         start=(d == 0), stop=(d == ND - 1))
                    rope(xk_ps, xk_sb[p], sb)

            # xv projection (natural [l, e] layout) + ones columns
            for lt in range(NLT):
                xv_ps = psV.tile([128, 512], F32, tag="xvps", name="xvps")
                for d in range(ND):
                    vt = vt_pool.tile([128, 128], BF16, tag="vt", name="vt")
                    nc.sync.dma_start(
                        out=vt,
                        in_=vT[d * 128:(d + 1) * 128, lt * 128:(lt + 1) * 128])
                    nc.tensor.matmul(xv_ps, lhsT=vt, rhs=wv_t[d],
                                     start=(d == 0), stop=(d == ND - 1))
                dst = xv_sb[lt].rearrange("p (h c) -> p h c", c=HD + 1)
                src = xv_ps.rearrange("p (h c) -> p h c", c=HD)
                nc.vector.tensor_copy(dst[:, :, 0:HD], src)
                nc.vector.memset(dst[:, :, HD], 1.0)

        # =============== Phase B: attention ===============
        with ExitStack() as phase_b:
            probs_pool = phase_b.enter_context(tc.tile_pool(name="probs", bufs=6))
            small_pool = phase_b.enter_context(tc.tile_pool(name="small", bufs=4))
            scB = phase_b.enter_context(
                tc.tile_pool(name="scB", bufs=2, space="PSUM"))
            pvP = phase_b.enter_context(
                tc.tile_pool(name="pvP", bufs=2, space="PSUM"))

            for p in range(NPAIR):
                for sb in range(NSB):
                    scol = slice(sb * 512, (sb + 1) * 512)
                    ha, hb = 2 * p, 2 * p + 1
                    pva = pvP.tile([128, 512], F32, tag="pv", name="pva")
                    pvb = pvP.tile([128, 512], F32, tag="pv", name="pvb")
                    # banks 0..31 = (lt, head) with head inner; chunks of <=3
                    banks = [(i // 2, i % 2) for i in range(2 * NLT)]
                    ci = 0
                    while ci < len(banks):
                        chunk = banks[ci:ci + 3]
                        ci += 3
                        sc = scB.tile([128, 512 * 3], F32, tag="sc", name="sc")
                        for j, (lt, lh) in enumerate(chunk):
                            hrow = slice(lh * 64, (lh + 1) * 64)
                            nc.tensor.matmul(
                                sc[:, j * 512:(j + 1) * 512],
                                lhsT=xk_sb[p][hrow, lt * 128:(lt + 1) * 128],
                                rhs=xq_sb[p][hrow, scol],
                                start=True, stop=True)
                        pr = probs_pool.tile([128, 512 * 3], BF16,
                                             tag="pr", name="pr")
                        nc.scalar.activation(
                            pr[:, 0:512 * len(chunk)], sc[:, 0:512 * len(chunk)],
                            Exp, scale=0.125)
                        for j, (lt, lh) in enumerate(chunk):
                            pvt = pva if lh == 0 else pvb
                            h = ha if lh == 0 else hb
                            nc.tensor.matmul(
                                pvt[0:HD + 1, :],
                                lhsT=xv_sb[lt][:, 65 * h:65 * h + 65],
                                rhs=pr[:, j * 512:(j + 1) * 512],
                                start=(lt == 0), stop=(lt == NLT - 1))
                    for (pvt, lh) in ((pva, 0), (pvb, 1)):
                        hrow = slice(lh * 64, (lh + 1) * 64)
                        den = small_pool.tile([1, 512], F32, tag="den", name="den")
                        nc.vector.tensor_copy(den, pvt[HD:HD + 1, :])
                        rc = small_pool.tile([1, 512], F32, tag="rc", name="rc")
                        nc.vector.reciprocal_approx_fast(out=rc, in_=den)
                        rb = small_pool.tile([64, 512], F32, tag="rb", name="rb")
                        nc.gpsimd.partition_broadcast(rb, rc)
                        nc.vector.tensor_mul(attT[p][hrow, scol],
                                             pvt[0:HD, :], rb)

        # =============== Phase C: output projection ===============
        with ExitStack() as phase_c:
            outp = phase_c.enter_context(tc.tile_pool(name="outp", bufs=3))
            psC = phase_c.enter_context(
                tc.tile_pool(name="psC", bufs=2, space="PSUM"))
            for st in range(NST):
                trow = slice(st * 128, (st + 1) * 128)
                ps = psC.tile([128, D], F32, tag="wops", name="wops")
                for et in range(NPAIR):
                    for nb in range(2):
                        nc.tensor.matmul(
                            ps[:, nb * 512:(nb + 1) * 512],
                            lhsT=attT[et][:, trow],
                            rhs=wo_t[et][:, nb * 512:(nb + 1) * 512],
                            start=(et == 0), stop=(et == NPAIR - 1))
                ot = outp.tile([128, D], F32, tag="ot", name="ot")
                nc.vector.tensor_copy(ot, ps)
                nc.sync.dma_start(out=out[trow, :], in_=ot)


def build_program():
    nc = bacc.Bacc("TRN2", target_bir_lowering=False, debug=False)
    qT = nc.dram_tensor("qT", [D, S], BF16, kind="ExternalInput").ap()
    kT = nc.dram_tensor("kT", [D, S], BF16, kind="ExternalInput").ap()
    vT = nc.dram_tensor("vT", [D, S], BF16, kind="ExternalInput").ap()
    wqT = nc.dram_tensor("wqT", [D, E], BF16, kind="ExternalInput").ap()
    wkT = nc.dram_tensor("wkT", [D, E], BF16, kind="ExternalInput").ap()
    wvT = nc.dram_tensor("wvT", [D, E], BF16, kind="ExternalInput").ap()
    woT = nc.dram_tensor("woT", [E, D], BF16, kind="ExternalInput").ap()
    ctab = nc.dram_tensor("ct", [128, S], F32, kind="ExternalInput").ap()
    stab = nc.dram_tensor("st", [128, S], F32, kind="ExternalInput").ap()
    out = nc.dram_tensor("out", [S, D], F32, kind="ExternalOutput").ap()
    aps = (qT, kT, vT, wqT, wkT, wvT, woT, ctab, stab, out)
    with tile.TileContext(nc) as tc:
        _emit(nc, tc, aps)
    nc.compile()
    return nc


def host_prep(q, k, v, freqs_cis, wq, wk, wv, wo):
    """Build the 8 per-core input maps."""
    q = np.asarray(q, dtype=np.float32)
    k = np.asarray(k, dtype=np.float32)
    v = np.asarray(v, dtype=np.float32)
    fc = np.asarray(freqs_cis, dtype=np.float32)
    wq = np.asarray(wq, dtype=np.float32)
    wk = np.asarray(wk, dtype=np.float32)
    wv = np.asarray(wv, dtype=np.float32)
    wo = np.asarray(wo, dtype=np.float32)

    cos, sin = fc[:, :, 0], fc[:, :, 1]            # (S, 32)
    idx = (np.arange(128) % 64) // 2
    ct = np.ascontiguousarray(cos[:, idx].T)       # (128, S)
    sgn = np.where(np.arange(128) % 2 == 0, -1.0, 1.0).astype(np.float32)
    st = np.ascontiguousarray(sin[:, idx].T * sgn[:, None])

    def b16(a):
        return np.ascontiguousarray(a).astype(NPBF16)

    in_maps = []
    for c in range(NCORES):
        b, g = c // 2, c % 2
        rows = slice(g * E, (g + 1) * E)
        in_maps.append({
            "qT": b16(q[:, b, :].T),
            "kT": b16(k[:, b, :].T),
            "vT": b16(v[:, b, :].T),
            "wqT": b16(wq[rows, :].T),
            "wkT": b16(wk[rows, :].T),
            "wvT": b16(wv[rows, :].T),
            "woT": b16(wo[:, rows].T),
            "ct": ct,
            "st": st,
        })
    return in_maps


def kernel(q, k, v, freqs_cis, wq, wk, wv, wo, trace=False):
    global _PROG, LAST_RESULT
    if _PROG is None:
        _PROG = build_program()
    in_maps = host_prep(q, k, v, freqs_cis, wq, wk, wv, wo)
    res = run_bass_kernel_spmd(_PROG, in_maps, list(range(NCORES)), trace=trace)
    LAST_RESULT = res
    out = np.empty((S, B, D), dtype=np.float32)
    for b in range(B):
        out[:, b, :] = res.results[2 * b]["out"] + res.results[2 * b + 1]["out"]
    return out
